# revision 1
# baseline (speedup 1.0000x reference)
"""Trainium2 Bass kernel for nn_AttentionToVec (B=8, N=4096, E=1024, H=16, D=64).

Strategy: data-parallel over batch (1 batch element per NeuronCore) for the
attention part; tensor-parallel over the MLP hidden dim (4096/8=512 per core)
with an AllGather of the per-core sampled vectors and a ReduceScatter of the
partial MLP outputs (which lands exactly each core's own output row).

Algebraic restructuring (host does weight-only folding):
  - att logits = x @ w_att where w_att[e,h] = sum_d W_k[e, h*D+d] * query[h,d]
    (the k-projection bias cancels inside softmax over n).
  - y[h,:] = sum_n softmax_att[n,h] * x[n,:]  (deferred 1/Z normalization)
  - sampled[h,d] = (y[h,:] @ W_v[:, h*D+d]) + b_v[h*D+d]   (sum_n att = 1)
"""

import numpy as np

B = 8
N = 4096
E = 1024
H = 16
D = 64
HID = 4096
NCORES = 8
HID_C = HID // NCORES

# Matmul operand dtype knobs per stage: "f32" | "f32r" | "bf16"
DT_ATT = "bf16"   # step 1: attT = w_attT @ xT
DT_Y = "f32r"     # step 3: y = exp_attT @ x (+ Z)
DT_V = "f32r"     # step 4: y @ W_v
DT_MLP = "f32r"   # MLP matmuls

_CACHE = {}


def _np_dt(knob):
    if knob == "bf16":
        import ml_dtypes

        return np.dtype(ml_dtypes.bfloat16)
    return np.dtype(np.float32)


def _build():
    import concourse.bacc as bacc
    import concourse.mybir as mybir
    from concourse import tile
    from concourse.masks import make_identity

    f32 = mybir.dt.float32
    bf16 = mybir.dt.bfloat16
    f32r = mybir.dt.float32r
    Act = mybir.ActivationFunctionType
    Alu = mybir.AluOpType

    def store_dt(knob):
        if knob == "bf16":
            return bf16
        if knob == "f32r":
            return f32r
        return f32

    def mm_ap(ap, knob):
        # tiles are already declared in the matmul dtype
        return ap

    nc = bacc.Bacc(None, target_bir_lowering=False, debug=True, num_devices=NCORES)

    dt_att = store_dt(DT_ATT)
    dt_y = store_dt(DT_Y)
    dt_v = store_dt(DT_V)
    dt_mlp = store_dt(DT_MLP)

    xT = nc.dram_tensor("xT", [E, N], dt_att, kind="ExternalInput")
    x = nc.dram_tensor("x", [N, E], dt_y, kind="ExternalInput")
    watt = nc.dram_tensor("watt", [E, H], dt_att, kind="ExternalInput")
    amask = nc.dram_tensor("amask", [H, N], f32, kind="ExternalInput")
    Wv = nc.dram_tensor("Wv", [E, E], dt_v, kind="ExternalInput")
    bvb = nc.dram_tensor("bvb", [H, E], f32, kind="ExternalInput")
    W1c = nc.dram_tensor("W1c", [E, HID_C], dt_mlp, kind="ExternalInput")
    b1c = nc.dram_tensor("b1c", [NCORES, HID_C], f32, kind="ExternalInput")
    W2c = nc.dram_tensor("W2c", [HID_C, E], dt_mlp, kind="ExternalInput")
    b2r8 = nc.dram_tensor("b2r8", [NCORES, E], f32, kind="ExternalInput")
    ones2 = nc.dram_tensor("ones2", [128, 2], dt_y, kind="ExternalInput")
    out = nc.dram_tensor("out", [1, E], f32, kind="ExternalOutput")

    with tile.TileContext(nc) as tc:
        with (
            tc.tile_pool(name="consts", bufs=1) as consts,
            tc.tile_pool(name="xtp", bufs=2) as xtp,
            tc.tile_pool(name="xp", bufs=4) as xp,
            tc.tile_pool(name="wvp", bufs=2) as wvp,
            tc.tile_pool(name="wmlp", bufs=1) as wmlp,
            tc.tile_pool(name="work", bufs=1) as work,
            tc.tile_pool(name="dramp", bufs=1, space="DRAM") as dramp,
        ):
            identity = consts.tile([128, 128], f32)
            make_identity(nc, identity[:])
            ones_col = consts.tile([128, 2], dt_y)
            nc.sync.dma_start(out=ones_col[:], in_=ones2[:, :])

            watt_s = consts.tile([128, 8, H], dt_att)
            nc.sync.dma_start(
                out=watt_s[:], in_=watt.ap().rearrange("(c p) h -> p c h", p=128)
            )
            amask_s = consts.tile([H, N], f32)
            nc.sync.dma_start(out=amask_s[:], in_=amask[:, :])
            bvb_s = consts.tile([H, E], f32)
            nc.sync.dma_start(out=bvb_s[:], in_=bvb[:, :])
            b1_s = consts.tile([NCORES, HID_C], f32)
            nc.sync.dma_start(out=b1_s[:], in_=b1c[:, :])
            b28_s = consts.tile([NCORES, E], f32)
            nc.sync.dma_start(out=b28_s[:], in_=b2r8[:, :])

            # ---- Phase A: attT[16, N] = w_att^T @ x^T (accumulate over e) ----
            psA_cm = tc.tile_pool(name="psA", bufs=1, space="PSUM")
            psA = psA_cm.__enter__()
            attT = psA.tile([H, N], f32)
            for c in range(8):
                xt = xtp.tile([128, N], dt_att, tag="xT")
                nc.sync.dma_start(out=xt[:], in_=xT[128 * c : 128 * (c + 1), :])
                for j in range(8):
                    nc.tensor.matmul(
                        attT[:, 512 * j : 512 * (j + 1)],
                        mm_ap(watt_s[:, c, :], DT_ATT),
                        mm_ap(xt[:, 512 * j : 512 * (j + 1)], DT_ATT),
                        start=(c == 0),
                        stop=(c == 7),
                    )

            # masked logits -> SBUF
            attm = work.tile([H, N], f32)
            for j in range(8):
                sl = slice(512 * j, 512 * (j + 1))
                nc.vector.tensor_add(attm[:, sl], attT[:, sl], amask_s[:, sl])
            psA_cm.__exit__(None, None, None)
            psTr_cm = tc.tile_pool(name="psTr", bufs=4, space="PSUM")
            psTr = psTr_cm.__enter__()
            psB_cm = tc.tile_pool(name="psB", bufs=1, space="PSUM")
            psB = psB_cm.__enter__()

            # ---- Phase A2 + B fused: per n-tile transpose+exp, then y/Z accum ----
            att_n = work.tile([128, 32 * H], dt_y)
            for t in range(32):
                tr = psTr.tile([128, H], f32, tag="tr")
                nc.tensor.transpose(
                    tr[:], attm[:, 128 * t : 128 * (t + 1)], identity[:H, :H]
                )
                nc.scalar.activation(att_n[:, H * t : H * (t + 1)], tr[:], Act.Exp)

            y_ps = psB.tile([H, E], f32, tag="acc")
            z_ps = psB.tile([H, 2], f32, tag="accz")
            xr = x.ap().rearrange("(tt u p) e -> tt p u e", u=2, p=128)
            for tt in range(16):
                xt2 = xp.tile([128, 2, E], dt_y, tag="x")
                nc.sync.dma_start(out=xt2[:], in_=xr[tt])
                for u in range(2):
                    t = 2 * tt + u
                    lhs = mm_ap(att_n[:, H * t : H * (t + 1)], DT_Y)
                    nc.tensor.matmul(
                        y_ps[:, 0:512],
                        lhs,
                        mm_ap(xt2[:, u, 0:512], DT_Y),
                        start=(t == 0),
                        stop=(t == 31),
                    )
                    nc.tensor.matmul(
                        y_ps[:, 512:1024],
                        lhs,
                        mm_ap(xt2[:, u, 512:1024], DT_Y),
                        start=(t == 0),
                        stop=(t == 31),
                    )
                    nc.tensor.matmul(
                        z_ps[:],
                        lhs,
                        mm_ap(ones_col[:], DT_Y),
                        start=(t == 0),
                        stop=(t == 31),
                    )

            rz = work.tile([H, 1], f32)
            nc.vector.reciprocal(rz[:], z_ps[:, 0:1])
            y_s = work.tile([H, E], f32)
            nc.vector.tensor_scalar_mul(y_s[:], y_ps[:], rz[:])

            # ---- Phase C: sampled = diag_blocks(y @ Wv) + bv ----
            yT = work.tile([128, 8 * H], dt_v)
            for j in range(8):
                tr2 = psTr.tile([128, H], f32, tag="tr")
                nc.tensor.transpose(
                    tr2[:], y_s[:, 128 * j : 128 * (j + 1)], identity[:H, :H]
                )
                nc.vector.tensor_copy(yT[:, H * j : H * (j + 1)], tr2[:])

            sf_ps = psB.tile([H, E], f32, tag="acc")
            for c in range(8):
                wv_t = wvp.tile([128, E], dt_v, tag="wv")
                nc.sync.dma_start(out=wv_t[:], in_=Wv[128 * c : 128 * (c + 1), :])
                for j in range(2):
                    nc.tensor.matmul(
                        sf_ps[:, 512 * j : 512 * (j + 1)],
                        mm_ap(yT[:, H * c : H * (c + 1)], DT_V),
                        mm_ap(wv_t[:, 512 * j : 512 * (j + 1)], DT_V),
                        start=(c == 0),
                        stop=(c == 7),
                    )

            # sampled[h, d] = sf[h, h*D + d] (bias already folded in).
            # Bounce sf through DRAM with padded rows; the AllGather input is a
            # skewed strided view that picks exactly the diagonal blocks.
            sf_s = work.tile([H, E], f32)
            nc.vector.tensor_add(sf_s[:], sf_ps[:], bvb_s[:])
            sf_d = dramp.tile([H, E + D], f32)
            nc.sync.dma_start(out=sf_d[:, :E], in_=sf_s[:])
            import concourse.bass as bass_mod

            sfd_ap = sf_d[:]
            diag_view = bass_mod.AP(
                tensor=sfd_ap.tensor, offset=0, ap=[[E + 2 * D, H], [1, D]]
            )
            s_loc = work.tile([H, D], f32)
            nc.sync.dma_start(out=s_loc[:], in_=diag_view)
            s_dram = dramp.tile([1, E], f32)
            nc.sync.dma_start(
                out=s_dram[:].rearrange("o (h d) -> (o h) d", h=H), in_=s_loc[:]
            )

            # ---- Phase D: AllGather sampled vectors ----
            S_all = dramp.tile([NCORES, E], f32, addr_space="Shared")
            nc.gpsimd.collective_compute(
                "AllGather",
                Alu.bypass,
                replica_groups=[list(range(NCORES))],
                ins=[s_dram[:].opt()],
                outs=[S_all[:].opt()],
            )

            S_s = work.tile([NCORES, E], f32)
            nc.sync.dma_start(out=S_s[:], in_=S_all[:])
            ST = work.tile([128, 8 * NCORES], dt_mlp)
            for j in range(8):
                tr3 = psTr.tile([128, H], f32, tag="tr")
                nc.tensor.transpose(
                    tr3[:, :NCORES],
                    S_s[:, 128 * j : 128 * (j + 1)],
                    identity[:NCORES, :NCORES],
                )
                nc.vector.tensor_copy(
                    ST[:, NCORES * j : NCORES * (j + 1)], tr3[:, :NCORES]
                )

            # ---- Phase E: MLP (tensor-parallel over hidden slice) ----
            w1_s = wmlp.tile([128, 8, HID_C], dt_mlp, tag="w1")
            nc.sync.dma_start(
                out=w1_s[:], in_=W1c.ap().rearrange("(c p) m -> p c m", p=128)
            )
            h1_ps = psB.tile([NCORES, HID_C], f32, tag="accz")
            for c in range(8):
                nc.tensor.matmul(
                    h1_ps[:],
                    mm_ap(ST[:, NCORES * c : NCORES * (c + 1)], DT_MLP),
                    mm_ap(w1_s[:, c, :], DT_MLP),
                    start=(c == 0),
                    stop=(c == 7),
                )

            # z = h1 + b1 ; gelu (tanh approx, matches jax.nn.gelu default)
            z_s = work.tile([NCORES, HID_C], f32)
            nc.vector.tensor_add(z_s[:], h1_ps[:], b1_s[:])
            sq = work.tile([NCORES, HID_C], f32, tag="ga")
            nc.scalar.activation(sq[:], z_s[:], Act.Square)
            cube = work.tile([NCORES, HID_C], f32, tag="gb")
            nc.vector.tensor_mul(cube[:], sq[:], z_s[:])
            uu = work.tile([NCORES, HID_C], f32, tag="ga")
            nc.vector.scalar_tensor_tensor(
                uu[:], cube[:], 0.044715, z_s[:], Alu.mult, Alu.add
            )
            th = work.tile([NCORES, HID_C], f32, tag="gb")
            nc.scalar.activation(th[:], uu[:], Act.Tanh, scale=0.7978845608028654)
            hh2 = work.tile([NCORES, HID_C], f32, tag="ga")
            nc.vector.scalar_tensor_tensor(
                hh2[:], th[:], 1.0, z_s[:], Alu.add, Alu.mult
            )
            nc.vector.tensor_scalar_mul(hh2[:], hh2[:], 0.5)

            hT = work.tile([128, 4 * NCORES], dt_mlp)
            for j in range(4):
                tr4 = psTr.tile([128, H], f32, tag="tr")
                nc.tensor.transpose(
                    tr4[:, :NCORES],
                    hh2[:, 128 * j : 128 * (j + 1)],
                    identity[:NCORES, :NCORES],
                )
                nc.vector.tensor_copy(
                    hT[:, NCORES * j : NCORES * (j + 1)], tr4[:, :NCORES]
                )

            w2_s = wmlp.tile([128, 4, E], dt_mlp, tag="w2")
            nc.sync.dma_start(
                out=w2_s[:], in_=W2c.ap().rearrange("(c p) e -> p c e", p=128)
            )
            p2_ps = psB.tile([NCORES, E], f32, tag="acc")
            for c in range(4):
                for j in range(2):
                    nc.tensor.matmul(
                        p2_ps[:, 512 * j : 512 * (j + 1)],
                        mm_ap(hT[:, NCORES * c : NCORES * (c + 1)], DT_MLP),
                        mm_ap(w2_s[:, c, 512 * j : 512 * (j + 1)], DT_MLP),
                        start=(c == 0),
                        stop=(c == 3),
                    )

            sb8 = work.tile([NCORES, E], f32)
            nc.vector.scalar_tensor_tensor(
                sb8[:], S_s[:], 0.125, b28_s[:], Alu.mult, Alu.add
            )
            mlp_s = work.tile([NCORES, E], f32)
            nc.vector.tensor_add(mlp_s[:], p2_ps[:], sb8[:])
            mlp_d = dramp.tile([NCORES, E], f32)
            nc.sync.dma_start(out=mlp_d[:], in_=mlp_s[:])

            # ---- Phase F: ReduceScatter -> this core's output row ----
            mlp_row = dramp.tile([1, E], f32)
            nc.gpsimd.collective_compute(
                "ReduceScatter",
                Alu.add,
                replica_groups=[list(range(NCORES))],
                ins=[mlp_d[:].opt()],
                outs=[mlp_row[:].opt()],
            )

            m_row = work.tile([1, E], f32)
            nc.sync.dma_start(out=m_row[:], in_=mlp_row[:])
            nc.sync.dma_start(out=out[:, :], in_=m_row[:])
            psB_cm.__exit__(None, None, None)
            psTr_cm.__exit__(None, None, None)

    return nc


def get_nc():
    if "nc" not in _CACHE:
        nc = _build()
        nc.finalize()
        _CACHE["nc"] = nc
    return _CACHE["nc"]


def build_in_maps(x, mask, W_kv, b_kv, query, W1, b1, W2, b2):
    """Host-side shard prep. Weight-only algebra + layout transforms."""
    x = np.asarray(x, np.float32)
    mask = np.asarray(mask)
    W_kv = np.asarray(W_kv, np.float32)
    b_kv = np.asarray(b_kv, np.float32)
    query = np.asarray(query, np.float32)
    W1 = np.asarray(W1, np.float32)
    b1 = np.asarray(b1, np.float32)
    W2 = np.asarray(W2, np.float32)
    b2 = np.asarray(b2, np.float32)

    W_k = W_kv[:, :E]
    W_v = W_kv[:, E:]
    # fold the per-head query into the k-projection: [E, H]
    w_att = np.einsum("ehd,hd->eh", W_k.reshape(E, H, D), query).astype(np.float32)
    bv_b = np.ascontiguousarray(np.broadcast_to(b_kv[None, E:], (H, E)).astype(np.float32))

    addmask = np.where(mask[:, :, 0], np.float32(-1e30), np.float32(0.0))  # [B, N]

    dt_att = _np_dt(DT_ATT)
    dt_y = _np_dt(DT_Y)
    dt_v = _np_dt(DT_V)
    dt_mlp = _np_dt(DT_MLP)

    Wv_c = np.ascontiguousarray(W_v.astype(dt_v))
    watt_c = np.ascontiguousarray(w_att.astype(dt_att))
    b2r8 = np.ascontiguousarray(np.broadcast_to(b2[None, :] / 8.0, (NCORES, E)).astype(np.float32))

    in_maps = []
    for c in range(NCORES):
        hs = slice(HID_C * c, HID_C * (c + 1))
        in_maps.append(
            {
                "xT": np.ascontiguousarray(x[c].T.astype(dt_att)),
                "x": np.ascontiguousarray(x[c].astype(dt_y)),
                "watt": watt_c,
                "amask": np.ascontiguousarray(
                    np.broadcast_to(addmask[c][None, :], (H, N))
                ),
                "Wv": Wv_c,
                "bvb": bv_b,
                "W1c": np.ascontiguousarray(W1[:, hs].astype(dt_mlp)),
                "b1c": np.ascontiguousarray(np.broadcast_to(b1[hs][None, :], (NCORES, HID_C))),
                "W2c": np.ascontiguousarray(W2[hs, :].astype(dt_mlp)),
                "b2r8": b2r8,
                "ones2": np.ones((128, 2), dtype=dt_y),
            }
        )
    return in_maps


def kernel(**inputs):
    from concourse.bass_utils import run_bass_kernel_spmd

    in_maps = build_in_maps(**inputs)
    nc = get_nc()
    res = run_bass_kernel_spmd(nc, in_maps, list(range(NCORES)), trace=False)
    return np.stack([res.results[c]["out"][0] for c in range(NCORES)]).astype(
        np.float32
    )



# revision 8
# speedup vs baseline: 1.0746x; 1.0746x over previous
"""Trainium2 Bass kernel for nn_AttentionToVec (B=8, N=4096, E=1024, H=16, D=64).

Strategy: data-parallel over batch (1 batch element per NeuronCore) for the
attention part; tensor-parallel over the MLP hidden dim (4096/8=512 per core)
with an AllGather of the per-core sampled vectors and a ReduceScatter of the
partial MLP outputs (which lands exactly each core's own output row).

Algebraic restructuring (host does weight-only folding):
  - att logits = x @ w_att where w_att[e,h] = sum_d W_k[e, h*D+d] * query[h,d]
    (the k-projection bias cancels inside softmax over n).
  - y[h,:] = sum_n softmax_att[n,h] * x[n,:]  (deferred 1/Z normalization)
  - sampled[h,d] = (y[h,:] @ W_v[:, h*D+d]) + b_v[h*D+d]   (sum_n att = 1)
"""

import numpy as np

B = 8
N = 4096
E = 1024
H = 16
D = 64
HID = 4096
NCORES = 8
HID_C = HID // NCORES

# Matmul operand dtype knobs per stage: "f32" | "f32r" | "bf16"
DT_ATT = "bf16"   # step 1: attT = w_attT @ xT
DT_Y = "bf16"     # step 3: y = exp_attT @ x (+ Z)
DT_V = "bf16"     # step 4: y @ W_v
DT_MLP = "bf16"   # MLP matmuls

_CACHE = {}


def _np_dt(knob):
    if knob == "bf16":
        import ml_dtypes

        return np.dtype(ml_dtypes.bfloat16)
    return np.dtype(np.float32)


def _build():
    import concourse.bacc as bacc
    import concourse.mybir as mybir
    from concourse import tile
    from concourse.masks import make_identity

    f32 = mybir.dt.float32
    bf16 = mybir.dt.bfloat16
    f32r = mybir.dt.float32r
    Act = mybir.ActivationFunctionType
    Alu = mybir.AluOpType

    def store_dt(knob):
        if knob == "bf16":
            return bf16
        if knob == "f32r":
            return f32r
        return f32

    def mm_ap(ap, knob):
        # tiles are already declared in the matmul dtype
        return ap

    nc = bacc.Bacc(None, target_bir_lowering=False, debug=True, num_devices=NCORES)

    dt_att = store_dt(DT_ATT)
    dt_y = store_dt(DT_Y)
    dt_v = store_dt(DT_V)
    dt_mlp = store_dt(DT_MLP)

    xT = nc.dram_tensor("xT", [E, N], dt_att, kind="ExternalInput")
    x = nc.dram_tensor("x", [N, E], dt_y, kind="ExternalInput")
    watt = nc.dram_tensor("watt", [E, H], dt_att, kind="ExternalInput")
    amask = nc.dram_tensor("amask", [H, N], f32, kind="ExternalInput")
    Wv = nc.dram_tensor("Wv", [E, E], dt_v, kind="ExternalInput")
    bvb = nc.dram_tensor("bvb", [H, E], f32, kind="ExternalInput")
    W1c = nc.dram_tensor("W1c", [E, HID_C], dt_mlp, kind="ExternalInput")
    b1c = nc.dram_tensor("b1c", [NCORES, HID_C], f32, kind="ExternalInput")
    W2c = nc.dram_tensor("W2c", [HID_C, E], dt_mlp, kind="ExternalInput")
    b2r8 = nc.dram_tensor("b2r8", [NCORES, E], f32, kind="ExternalInput")
    ones2 = nc.dram_tensor("ones2", [128, 2], dt_y, kind="ExternalInput")
    out = nc.dram_tensor("out", [1, E], f32, kind="ExternalOutput")

    with tile.TileContext(nc) as tc:
        with (
            tc.tile_pool(name="consts", bufs=1) as consts,
            tc.tile_pool(name="xtp", bufs=2) as xtp,
            tc.tile_pool(name="xp", bufs=4) as xp,
            tc.tile_pool(name="wvp", bufs=2) as wvp,
            tc.tile_pool(name="wmlp", bufs=1) as wmlp,
            tc.tile_pool(name="work", bufs=1) as work,
            tc.tile_pool(name="dramp", bufs=1, space="DRAM") as dramp,
        ):
            identity = consts.tile([128, 128], f32)
            make_identity(nc, identity[:])
            ones_col = consts.tile([128, 2], dt_y)
            nc.sync.dma_start(out=ones_col[:], in_=ones2[:, :])

            watt_s = consts.tile([128, 8, H], dt_att)
            nc.sync.dma_start(
                out=watt_s[:], in_=watt.ap().rearrange("(c p) h -> p c h", p=128)
            )
            amask_s = consts.tile([H, N], f32)
            nc.sync.dma_start(out=amask_s[:], in_=amask[:, :])
            bvb_s = consts.tile([H, E], f32)
            nc.sync.dma_start(out=bvb_s[:], in_=bvb[:, :])
            b1_s = consts.tile([NCORES, HID_C], f32)
            nc.sync.dma_start(out=b1_s[:], in_=b1c[:, :])
            b28_s = consts.tile([NCORES, E], f32)
            nc.sync.dma_start(out=b28_s[:], in_=b2r8[:, :])

            # ---- Phase A: attT[16, N] = w_att^T @ x^T (accumulate over e) ----
            psA_cm = tc.tile_pool(name="psA", bufs=1, space="PSUM")
            psA = psA_cm.__enter__()
            attT = psA.tile([H, N], f32)
            for c in range(8):
                xt = xtp.tile([128, N], dt_att, tag="xT")
                nc.sync.dma_start(out=xt[:], in_=xT[128 * c : 128 * (c + 1), :])
                for j in range(8):
                    nc.tensor.matmul(
                        attT[:, 512 * j : 512 * (j + 1)],
                        mm_ap(watt_s[:, c, :], DT_ATT),
                        mm_ap(xt[:, 512 * j : 512 * (j + 1)], DT_ATT),
                        start=(c == 0),
                        stop=(c == 7),
                    )

            # masked logits -> SBUF
            attm = work.tile([H, N], f32)
            for j in range(8):
                sl = slice(512 * j, 512 * (j + 1))
                nc.vector.tensor_add(attm[:, sl], attT[:, sl], amask_s[:, sl])
            psA_cm.__exit__(None, None, None)
            psTr_cm = tc.tile_pool(name="psTr", bufs=4, space="PSUM")
            psTr = psTr_cm.__enter__()
            psB_cm = tc.tile_pool(name="psB", bufs=1, space="PSUM")
            psB = psB_cm.__enter__()

            # ---- Phase A2 + B fused: per n-tile transpose+exp, then y/Z accum ----
            att_n = work.tile([128, 32 * H], dt_y)
            for t in range(32):
                tr = psTr.tile([128, H], f32, tag="tr")
                nc.tensor.transpose(
                    tr[:], attm[:, 128 * t : 128 * (t + 1)], identity[:H, :H]
                )
                nc.scalar.activation(att_n[:, H * t : H * (t + 1)], tr[:], Act.Exp)

            y_ps = psB.tile([H, E], f32, tag="acc")
            z_ps = psB.tile([H, 2], f32, tag="accz")
            xr = x.ap().rearrange("(tt u p) e -> tt p u e", u=2, p=128)
            for tt in range(16):
                xt2 = xp.tile([128, 2, E], dt_y, tag="x")
                nc.sync.dma_start(out=xt2[:], in_=xr[tt])
                for u in range(2):
                    t = 2 * tt + u
                    lhs = mm_ap(att_n[:, H * t : H * (t + 1)], DT_Y)
                    nc.tensor.matmul(
                        y_ps[:, 0:512],
                        lhs,
                        mm_ap(xt2[:, u, 0:512], DT_Y),
                        start=(t == 0),
                        stop=(t == 31),
                    )
                    nc.tensor.matmul(
                        y_ps[:, 512:1024],
                        lhs,
                        mm_ap(xt2[:, u, 512:1024], DT_Y),
                        start=(t == 0),
                        stop=(t == 31),
                    )
                    nc.tensor.matmul(
                        z_ps[:],
                        lhs,
                        mm_ap(ones_col[:], DT_Y),
                        start=(t == 0),
                        stop=(t == 31),
                    )

            rz = work.tile([H, 1], f32)
            nc.vector.reciprocal(rz[:], z_ps[:, 0:1])
            y_s = work.tile([H, E], f32)
            nc.vector.tensor_scalar_mul(y_s[:], y_ps[:], rz[:])

            # ---- Phase C: sampled = diag_blocks(y @ Wv) + bv ----
            yT = work.tile([128, 8 * H], dt_v)
            for j in range(8):
                tr2 = psTr.tile([128, H], f32, tag="tr")
                nc.tensor.transpose(
                    tr2[:], y_s[:, 128 * j : 128 * (j + 1)], identity[:H, :H]
                )
                nc.vector.tensor_copy(yT[:, H * j : H * (j + 1)], tr2[:])

            sf_ps = psB.tile([H, E], f32, tag="acc")
            for c in range(8):
                wv_t = wvp.tile([128, E], dt_v, tag="wv")
                nc.sync.dma_start(out=wv_t[:], in_=Wv[128 * c : 128 * (c + 1), :])
                for j in range(2):
                    nc.tensor.matmul(
                        sf_ps[:, 512 * j : 512 * (j + 1)],
                        mm_ap(yT[:, H * c : H * (c + 1)], DT_V),
                        mm_ap(wv_t[:, 512 * j : 512 * (j + 1)], DT_V),
                        start=(c == 0),
                        stop=(c == 7),
                    )

            # sampled[h, d] = sf[h, h*D + d] (bias already folded in).
            # Write sf to DRAM padded rows; the AllGather input is a skewed
            # strided view that picks exactly the diagonal blocks.
            sf_s = work.tile([H, E], f32)
            nc.vector.tensor_add(sf_s[:], sf_ps[:], bvb_s[:])
            sf_d = dramp.tile([H, E + D], f32)
            nc.sync.dma_start(out=sf_d[:, :E], in_=sf_s[:])
            import concourse.bass as bass_mod

            sfd_ap = sf_d[:]
            diag_view = bass_mod.AP(
                tensor=sfd_ap.tensor, offset=0, ap=[[E + 2 * D, H], [1, D]]
            )
            s_loc = work.tile([H, D], f32)
            nc.sync.dma_start(out=s_loc[:], in_=diag_view)
            s_dram = dramp.tile([1, E], f32)
            nc.sync.dma_start(
                out=s_dram[:].rearrange("o (h d) -> (o h) d", h=H), in_=s_loc[:]
            )

            # ---- Phase D: AllGather sampled vectors ----
            S_all = dramp.tile([NCORES, E], f32, addr_space="Shared")
            nc.gpsimd.collective_compute(
                "AllGather",
                Alu.bypass,
                replica_groups=[list(range(NCORES))],
                ins=[s_dram[:].opt()],
                outs=[S_all[:].opt()],
            )

            S_s = work.tile([NCORES, E], f32)
            nc.sync.dma_start(out=S_s[:], in_=S_all[:])
            ST = work.tile([128, 8 * NCORES], dt_mlp)
            for j in range(8):
                tr3 = psTr.tile([128, H], f32, tag="tr")
                nc.tensor.transpose(
                    tr3[:, :NCORES],
                    S_s[:, 128 * j : 128 * (j + 1)],
                    identity[:NCORES, :NCORES],
                )
                nc.vector.tensor_copy(
                    ST[:, NCORES * j : NCORES * (j + 1)], tr3[:, :NCORES]
                )

            # ---- Phase E: MLP (tensor-parallel over hidden slice) ----
            w1_s = wmlp.tile([128, 8, HID_C], dt_mlp, tag="w1")
            nc.sync.dma_start(
                out=w1_s[:], in_=W1c.ap().rearrange("(c p) m -> p c m", p=128)
            )
            h1_ps = psB.tile([NCORES, HID_C], f32, tag="accz")
            for c in range(8):
                nc.tensor.matmul(
                    h1_ps[:],
                    mm_ap(ST[:, NCORES * c : NCORES * (c + 1)], DT_MLP),
                    mm_ap(w1_s[:, c, :], DT_MLP),
                    start=(c == 0),
                    stop=(c == 7),
                )

            # z = h1 + b1 ; gelu (tanh approx, matches jax.nn.gelu default)
            z_s = work.tile([NCORES, HID_C], f32)
            nc.vector.tensor_add(z_s[:], h1_ps[:], b1_s[:])
            hh2 = work.tile([NCORES, HID_C], f32, tag="ga")
            nc.scalar.activation(hh2[:], z_s[:], Act.Gelu_apprx_tanh)

            hT = work.tile([128, 4 * NCORES], dt_mlp)
            for j in range(4):
                tr4 = psTr.tile([128, H], f32, tag="tr")
                nc.tensor.transpose(
                    tr4[:, :NCORES],
                    hh2[:, 128 * j : 128 * (j + 1)],
                    identity[:NCORES, :NCORES],
                )
                nc.vector.tensor_copy(
                    hT[:, NCORES * j : NCORES * (j + 1)], tr4[:, :NCORES]
                )

            w2_s = wmlp.tile([128, 4, E], dt_mlp, tag="w2")
            nc.sync.dma_start(
                out=w2_s[:], in_=W2c.ap().rearrange("(c p) e -> p c e", p=128)
            )
            p2_ps = psB.tile([NCORES, E], f32, tag="acc")
            for c in range(4):
                for j in range(2):
                    nc.tensor.matmul(
                        p2_ps[:, 512 * j : 512 * (j + 1)],
                        mm_ap(hT[:, NCORES * c : NCORES * (c + 1)], DT_MLP),
                        mm_ap(w2_s[:, c, 512 * j : 512 * (j + 1)], DT_MLP),
                        start=(c == 0),
                        stop=(c == 3),
                    )

            sb8 = work.tile([NCORES, E], f32)
            nc.vector.scalar_tensor_tensor(
                sb8[:], S_s[:], 0.125, b28_s[:], Alu.mult, Alu.add
            )
            mlp_s = work.tile([NCORES, E], f32)
            nc.vector.tensor_add(mlp_s[:], p2_ps[:], sb8[:])
            mlp_d = dramp.tile([NCORES, E], f32)
            nc.sync.dma_start(out=mlp_d[:], in_=mlp_s[:])

            # ---- Phase F: ReduceScatter -> this core's output row ----
            mlp_row = dramp.tile([1, E], f32)
            nc.gpsimd.collective_compute(
                "ReduceScatter",
                Alu.add,
                replica_groups=[list(range(NCORES))],
                ins=[mlp_d[:].opt()],
                outs=[mlp_row[:].opt()],
            )

            nc.sync.dma_start(out=out[:, :], in_=mlp_row[:])
            psB_cm.__exit__(None, None, None)
            psTr_cm.__exit__(None, None, None)

    return nc


def get_nc():
    if "nc" not in _CACHE:
        nc = _build()
        nc.finalize()
        _CACHE["nc"] = nc
    return _CACHE["nc"]


def build_in_maps(x, mask, W_kv, b_kv, query, W1, b1, W2, b2):
    """Host-side shard prep. Weight-only algebra + layout transforms."""
    x = np.asarray(x, np.float32)
    mask = np.asarray(mask)
    W_kv = np.asarray(W_kv, np.float32)
    b_kv = np.asarray(b_kv, np.float32)
    query = np.asarray(query, np.float32)
    W1 = np.asarray(W1, np.float32)
    b1 = np.asarray(b1, np.float32)
    W2 = np.asarray(W2, np.float32)
    b2 = np.asarray(b2, np.float32)

    W_k = W_kv[:, :E]
    W_v = W_kv[:, E:]
    # fold the per-head query into the k-projection: [E, H]
    w_att = np.einsum("ehd,hd->eh", W_k.reshape(E, H, D), query).astype(np.float32)
    bv_b = np.ascontiguousarray(np.broadcast_to(b_kv[None, E:], (H, E)).astype(np.float32))

    addmask = np.where(mask[:, :, 0], np.float32(-1e30), np.float32(0.0))  # [B, N]

    dt_att = _np_dt(DT_ATT)
    dt_y = _np_dt(DT_Y)
    dt_v = _np_dt(DT_V)
    dt_mlp = _np_dt(DT_MLP)

    Wv_c = np.ascontiguousarray(W_v.astype(dt_v))
    watt_c = np.ascontiguousarray(w_att.astype(dt_att))
    b2r8 = np.ascontiguousarray(np.broadcast_to(b2[None, :] / 8.0, (NCORES, E)).astype(np.float32))

    in_maps = []
    for c in range(NCORES):
        hs = slice(HID_C * c, HID_C * (c + 1))
        in_maps.append(
            {
                "xT": np.ascontiguousarray(x[c].T.astype(dt_att)),
                "x": np.ascontiguousarray(x[c].astype(dt_y)),
                "watt": watt_c,
                "amask": np.ascontiguousarray(
                    np.broadcast_to(addmask[c][None, :], (H, N))
                ),
                "Wv": Wv_c,
                "bvb": bv_b,
                "W1c": np.ascontiguousarray(W1[:, hs].astype(dt_mlp)),
                "b1c": np.ascontiguousarray(np.broadcast_to(b1[hs][None, :], (NCORES, HID_C))),
                "W2c": np.ascontiguousarray(W2[hs, :].astype(dt_mlp)),
                "b2r8": b2r8,
                "ones2": np.ones((128, 2), dtype=dt_y),
            }
        )
    return in_maps


def kernel(**inputs):
    from concourse.bass_utils import run_bass_kernel_spmd

    in_maps = build_in_maps(**inputs)
    nc = get_nc()
    res = run_bass_kernel_spmd(nc, in_maps, list(range(NCORES)), trace=False)
    return np.stack([res.results[c]["out"][0] for c in range(NCORES)]).astype(
        np.float32
    )



# revision 10
# speedup vs baseline: 1.1449x; 1.0654x over previous
"""Trainium2 Bass kernel for nn_AttentionToVec (B=8, N=4096, E=1024, H=16, D=64).

Strategy: data-parallel over batch (1 batch element per NeuronCore) for the
attention part; tensor-parallel over the MLP hidden dim (4096/8=512 per core)
with an AllGather of the per-core sampled vectors and a ReduceScatter of the
partial MLP outputs (which lands exactly each core's own output row).

Algebraic restructuring (host does weight-only folding):
  - att logits = x @ w_att where w_att[e,h] = sum_d W_k[e, h*D+d] * query[h,d]
    (the k-projection bias cancels inside softmax over n).
  - y[h,:] = sum_n softmax_att[n,h] * x[n,:]  (deferred 1/Z normalization)
  - sampled[h,d] = (y[h,:] @ W_v[:, h*D+d]) + b_v[h*D+d]   (sum_n att = 1)

The attention stream is fused: the sequence is processed in 4 super-tiles of
1024 positions. Per super-tile: logits (from a host-pretransposed x^T copy),
mask+exp, and the y/Z accumulation (x with two appended ones-columns folds Z
into the same matmul). Software-pipelined so the PE stays busy across the
vector-add boundary. All weights are prefetched during the stream.
"""

import numpy as np

B = 8
N = 4096
E = 1024
H = 16
D = 64
HID = 4096
NCORES = 8
HID_C = HID // NCORES
NT = 4          # super-tiles over the sequence
TN = N // NT    # 1024 sequence positions per super-tile
EA = E + 2      # x augmented with two ones-columns (Z accumulator)

_CACHE = {}


def _bf16():
    import ml_dtypes

    return np.dtype(ml_dtypes.bfloat16)


def _build():
    import concourse.bacc as bacc
    import concourse.mybir as mybir
    from concourse import tile
    from concourse.masks import make_identity

    f32 = mybir.dt.float32
    bf16 = mybir.dt.bfloat16
    Act = mybir.ActivationFunctionType
    Alu = mybir.AluOpType

    nc = bacc.Bacc(None, target_bir_lowering=False, debug=True, num_devices=NCORES)

    # Host-prearranged layouts (see build_in_maps):
    #  xTt[T*128+p, c*1024+j] = x[T*1024+j, c*128+p]   (x^T in super-tile-major)
    #  xta[T*128+p, u*EA+e]   = x_aug[T*1024+u*128+p, e]
    xTt = nc.dram_tensor("xTt", [NT * 128, 8 * TN], bf16, kind="ExternalInput")
    xta = nc.dram_tensor("xta", [NT * 128, 8 * EA], bf16, kind="ExternalInput")
    watt = nc.dram_tensor("watt", [E, H], bf16, kind="ExternalInput")
    amask = nc.dram_tensor("amask", [H, N], f32, kind="ExternalInput")
    Wv = nc.dram_tensor("Wv", [E, E], bf16, kind="ExternalInput")
    bvb = nc.dram_tensor("bvb", [H, E], f32, kind="ExternalInput")
    W1c = nc.dram_tensor("W1c", [E, HID_C], bf16, kind="ExternalInput")
    b1c = nc.dram_tensor("b1c", [NCORES, HID_C], f32, kind="ExternalInput")
    W2c = nc.dram_tensor("W2c", [HID_C, E], bf16, kind="ExternalInput")
    b2r8 = nc.dram_tensor("b2r8", [NCORES, E], f32, kind="ExternalInput")
    out = nc.dram_tensor("out", [1, E], f32, kind="ExternalOutput")

    with tile.TileContext(nc) as tc:
        with (
            tc.tile_pool(name="consts", bufs=1) as consts,
            tc.tile_pool(name="xtp", bufs=2) as xtp,
            tc.tile_pool(name="xap", bufs=3) as xap,
            tc.tile_pool(name="wvp", bufs=1) as wvp,
            tc.tile_pool(name="wmlp", bufs=1) as wmlp,
            tc.tile_pool(name="attp", bufs=2) as attp,
            tc.tile_pool(name="attnp", bufs=2) as attnp,
            tc.tile_pool(name="work", bufs=1) as work,
            tc.tile_pool(name="dramp", bufs=1, space="DRAM") as dramp,
            tc.tile_pool(name="psA", bufs=1, space="PSUM") as psA,
            tc.tile_pool(name="psB", bufs=1, space="PSUM") as psB,
            tc.tile_pool(name="psTr", bufs=2, space="PSUM") as psTr,
        ):
            identity = consts.tile([128, 128], f32)
            make_identity(nc, identity[:])

            watt_s = consts.tile([128, 8, H], bf16)
            nc.sync.dma_start(
                out=watt_s[:], in_=watt.ap().rearrange("(c p) h -> p c h", p=128)
            )
            amask_s = consts.tile([H, N], f32)
            nc.sync.dma_start(out=amask_s[:], in_=amask[:, :])
            bvb_s = consts.tile([H, E], f32)
            nc.sync.dma_start(out=bvb_s[:], in_=bvb[:, :])
            b1_s = consts.tile([NCORES, HID_C], f32)
            nc.sync.dma_start(out=b1_s[:], in_=b1c[:, :])
            b28_s = consts.tile([NCORES, E], f32)
            nc.sync.dma_start(out=b28_s[:], in_=b2r8[:, :])

            # ---- Fused attention stream over 4 super-tiles ----
            y_ps = psB.tile([H, E], f32, tag="acc")
            z_ps = psB.tile([H, 2], f32, tag="accz")

            wv_s = None
            w1_s = None
            w2_s = None
            stage = {}  # T -> (attm, xa_sb)

            for T in range(NT + 1):
                if T < NT:
                    rs = slice(128 * T, 128 * (T + 1))
                    xT_sb = xtp.tile([128, 8, TN], bf16, tag="xT")
                    nc.sync.dma_start(out=xT_sb[:], in_=xTt[rs, :])
                    xa_sb = xap.tile([128, 8, EA], bf16, tag="xa")
                    nc.sync.dma_start(out=xa_sb[:], in_=xta[rs, :])
                    if T == 1:
                        # prefetch Wv during the stream
                        wv_s = wvp.tile([128, 8, E], bf16)
                        nc.sync.dma_start(
                            out=wv_s[:],
                            in_=Wv.ap().rearrange("(c p) e -> p c e", p=128),
                        )
                    if T == 2:
                        # prefetch MLP weights during the stream
                        w1_s = wmlp.tile([128, 8, HID_C], bf16, tag="w1")
                        nc.sync.dma_start(
                            out=w1_s[:],
                            in_=W1c.ap().rearrange("(c p) m -> p c m", p=128),
                        )
                        w2_s = wmlp.tile([128, 4, E], bf16, tag="w2")
                        nc.sync.dma_start(
                            out=w2_s[:],
                            in_=W2c.ap().rearrange("(c p) e -> p c e", p=128),
                        )

                    at_ps = psA.tile([H, TN], f32, tag="attT")
                    for c in range(8):
                        for j in range(2):
                            sl = slice(512 * j, 512 * (j + 1))
                            nc.tensor.matmul(
                                at_ps[:, sl],
                                watt_s[:, c, :],
                                xT_sb[:, c, sl],
                                start=(c == 0),
                                stop=(c == 7),
                            )
                    attm = attp.tile([H, TN], f32, tag="attm")
                    nc.vector.tensor_add(
                        attm[:], at_ps[:], amask_s[:, TN * T : TN * (T + 1)]
                    )
                    stage[T] = (attm, xa_sb)

                if T >= 1:
                    attm_p, xa_p = stage.pop(T - 1)
                    attn = attnp.tile([128, 8, H], bf16, tag="attn")
                    for u in range(8):
                        t = 8 * (T - 1) + u
                        tr = psTr.tile([128, H], f32, tag="tr")
                        nc.tensor.transpose(
                            tr[:], attm_p[:, 128 * u : 128 * (u + 1)], identity[:H, :H]
                        )
                        nc.scalar.activation(attn[:, u, :], tr[:], Act.Exp)
                        lhs = attn[:, u, :]
                        nc.tensor.matmul(
                            y_ps[:, 0:512],
                            lhs,
                            xa_p[:, u, 0:512],
                            start=(t == 0),
                            stop=(t == 31),
                        )
                        nc.tensor.matmul(
                            y_ps[:, 512:1024],
                            lhs,
                            xa_p[:, u, 512:1024],
                            start=(t == 0),
                            stop=(t == 31),
                        )
                        nc.tensor.matmul(
                            z_ps[:],
                            lhs,
                            xa_p[:, u, 1024:1026],
                            start=(t == 0),
                            stop=(t == 31),
                        )

            rz = work.tile([H, 1], f32)
            nc.vector.reciprocal(rz[:], z_ps[:, 0:1])
            y_s = work.tile([H, E], f32)
            nc.vector.tensor_scalar_mul(y_s[:], y_ps[:], rz[:])

            # ---- Phase C: sampled = diag_blocks(y @ Wv) + bv ----
            yT = work.tile([128, 8 * H], bf16)
            for j in range(8):
                tr2 = psTr.tile([128, H], f32, tag="tr")
                nc.tensor.transpose(
                    tr2[:], y_s[:, 128 * j : 128 * (j + 1)], identity[:H, :H]
                )
                nc.vector.tensor_copy(yT[:, H * j : H * (j + 1)], tr2[:])

            sf_ps = psB.tile([H, E], f32, tag="acc")
            for c in range(8):
                for j in range(2):
                    nc.tensor.matmul(
                        sf_ps[:, 512 * j : 512 * (j + 1)],
                        yT[:, H * c : H * (c + 1)],
                        wv_s[:, c, 512 * j : 512 * (j + 1)],
                        start=(c == 0),
                        stop=(c == 7),
                    )

            # sampled[h, d] = sf[h, h*D + d] (bias already folded in).
            # Bounce sf through DRAM with padded rows; the strided view picks
            # exactly the diagonal blocks.
            sf_s = work.tile([H, E], f32)
            nc.vector.tensor_add(sf_s[:], sf_ps[:], bvb_s[:])
            sf_d = dramp.tile([H, E + D], f32)
            nc.sync.dma_start(out=sf_d[:, :E], in_=sf_s[:])
            import concourse.bass as bass_mod

            sfd_ap = sf_d[:]
            diag_view = bass_mod.AP(
                tensor=sfd_ap.tensor, offset=0, ap=[[E + 2 * D, H], [1, D]]
            )
            s_loc = work.tile([H, D], f32)
            nc.sync.dma_start(out=s_loc[:], in_=diag_view)
            s_dram = dramp.tile([1, E], f32)
            nc.sync.dma_start(
                out=s_dram[:].rearrange("o (h d) -> (o h) d", h=H), in_=s_loc[:]
            )

            # ---- Phase D: AllGather sampled vectors ----
            S_all = dramp.tile([NCORES, E], f32, addr_space="Shared")
            nc.gpsimd.collective_compute(
                "AllGather",
                Alu.bypass,
                replica_groups=[list(range(NCORES))],
                ins=[s_dram[:].opt()],
                outs=[S_all[:].opt()],
            )

            S_s = work.tile([NCORES, E], f32)
            nc.sync.dma_start(out=S_s[:], in_=S_all[:])
            ST = work.tile([128, 8 * NCORES], bf16)
            for j in range(8):
                tr3 = psTr.tile([128, H], f32, tag="tr")
                nc.tensor.transpose(
                    tr3[:, :NCORES],
                    S_s[:, 128 * j : 128 * (j + 1)],
                    identity[:NCORES, :NCORES],
                )
                nc.vector.tensor_copy(
                    ST[:, NCORES * j : NCORES * (j + 1)], tr3[:, :NCORES]
                )

            # ---- Phase E: MLP (tensor-parallel over hidden slice) ----
            h1_ps = psB.tile([NCORES, HID_C], f32, tag="accz")
            for c in range(8):
                nc.tensor.matmul(
                    h1_ps[:],
                    ST[:, NCORES * c : NCORES * (c + 1)],
                    w1_s[:, c, :],
                    start=(c == 0),
                    stop=(c == 7),
                )

            # z = h1 + b1 ; gelu (tanh approx, matches jax.nn.gelu default)
            z_s = work.tile([NCORES, HID_C], f32)
            nc.vector.tensor_add(z_s[:], h1_ps[:], b1_s[:])
            hh2 = work.tile([NCORES, HID_C], f32, tag="ga")
            nc.scalar.activation(hh2[:], z_s[:], Act.Gelu_apprx_tanh)

            hT = work.tile([128, 4 * NCORES], bf16)
            for j in range(4):
                tr4 = psTr.tile([128, H], f32, tag="tr")
                nc.tensor.transpose(
                    tr4[:, :NCORES],
                    hh2[:, 128 * j : 128 * (j + 1)],
                    identity[:NCORES, :NCORES],
                )
                nc.vector.tensor_copy(
                    hT[:, NCORES * j : NCORES * (j + 1)], tr4[:, :NCORES]
                )

            p2_ps = psB.tile([NCORES, E], f32, tag="acc")
            for c in range(4):
                for j in range(2):
                    nc.tensor.matmul(
                        p2_ps[:, 512 * j : 512 * (j + 1)],
                        hT[:, NCORES * c : NCORES * (c + 1)],
                        w2_s[:, c, 512 * j : 512 * (j + 1)],
                        start=(c == 0),
                        stop=(c == 3),
                    )

            sb8 = work.tile([NCORES, E], f32)
            nc.vector.scalar_tensor_tensor(
                sb8[:], S_s[:], 0.125, b28_s[:], Alu.mult, Alu.add
            )
            mlp_s = work.tile([NCORES, E], f32)
            nc.vector.tensor_add(mlp_s[:], p2_ps[:], sb8[:])
            mlp_d = dramp.tile([NCORES, E], f32)
            nc.sync.dma_start(out=mlp_d[:], in_=mlp_s[:])

            # ---- Phase F: ReduceScatter -> this core's output row ----
            mlp_row = dramp.tile([1, E], f32)
            nc.gpsimd.collective_compute(
                "ReduceScatter",
                Alu.add,
                replica_groups=[list(range(NCORES))],
                ins=[mlp_d[:].opt()],
                outs=[mlp_row[:].opt()],
            )

            nc.sync.dma_start(out=out[:, :], in_=mlp_row[:])

    return nc


def get_nc():
    if "nc" not in _CACHE:
        nc = _build()
        nc.finalize()
        _CACHE["nc"] = nc
    return _CACHE["nc"]


def build_in_maps(x, mask, W_kv, b_kv, query, W1, b1, W2, b2):
    """Host-side shard prep. Weight-only algebra + layout transforms."""
    bf16 = _bf16()
    x = np.asarray(x, np.float32)
    mask = np.asarray(mask)
    W_kv = np.asarray(W_kv, np.float32)
    b_kv = np.asarray(b_kv, np.float32)
    query = np.asarray(query, np.float32)
    W1 = np.asarray(W1, np.float32)
    b1 = np.asarray(b1, np.float32)
    W2 = np.asarray(W2, np.float32)
    b2 = np.asarray(b2, np.float32)

    W_k = W_kv[:, :E]
    W_v = W_kv[:, E:]
    # fold the per-head query into the k-projection: [E, H]
    w_att = np.einsum("ehd,hd->eh", W_k.reshape(E, H, D), query).astype(np.float32)
    bv_b = np.ascontiguousarray(
        np.broadcast_to(b_kv[None, E:], (H, E)).astype(np.float32)
    )

    addmask = np.where(mask[:, :, 0], np.float32(-1e30), np.float32(0.0))  # [B, N]

    Wv_c = np.ascontiguousarray(W_v.astype(bf16))
    watt_c = np.ascontiguousarray(w_att.astype(bf16))
    b2r8 = np.ascontiguousarray(
        np.broadcast_to(b2[None, :] / 8.0, (NCORES, E)).astype(np.float32)
    )
    W1c_all = W1.astype(bf16)
    W2c_all = W2.astype(bf16)

    in_maps = []
    for c in range(NCORES):
        hs = slice(HID_C * c, HID_C * (c + 1))
        xb = x[c].astype(bf16)  # [N, E]
        # xTt[T, p, cc, j] = x[T*1024+j, cc*128+p]
        xTt = np.ascontiguousarray(
            xb.T.reshape(8, 128, NT, TN).transpose(2, 1, 0, 3).reshape(NT * 128, 8 * TN)
        )
        xa = np.concatenate([xb, np.ones((N, 2), dtype=bf16)], axis=1)  # [N, EA]
        # xta[T, p, u, e] = x_aug[T*1024 + u*128 + p, e]
        xta = np.ascontiguousarray(
            xa.reshape(NT, 8, 128, EA).transpose(0, 2, 1, 3).reshape(NT * 128, 8 * EA)
        )
        in_maps.append(
            {
                "xTt": xTt,
                "xta": xta,
                "watt": watt_c,
                "amask": np.ascontiguousarray(
                    np.broadcast_to(addmask[c][None, :], (H, N))
                ),
                "Wv": Wv_c,
                "bvb": bv_b,
                "W1c": np.ascontiguousarray(W1c_all[:, hs]),
                "b1c": np.ascontiguousarray(
                    np.broadcast_to(b1[hs][None, :], (NCORES, HID_C))
                ),
                "W2c": np.ascontiguousarray(W2c_all[hs, :]),
                "b2r8": b2r8,
            }
        )
    return in_maps


def kernel(**inputs):
    from concourse.bass_utils import run_bass_kernel_spmd

    in_maps = build_in_maps(**inputs)
    nc = get_nc()
    res = run_bass_kernel_spmd(nc, in_maps, list(range(NCORES)), trace=False)
    return np.stack([res.results[c]["out"][0] for c in range(NCORES)]).astype(
        np.float32
    )
